# revision 3
# baseline (speedup 1.0000x reference)
"""Trainium2 Bass kernel for nn_DCG_payoff (GNN message-passing payoff MLP).

Math (per batch b):
  hf = hidden[b, edges_from]; ht = hidden[b, edges_to]           # [E, DH]
  z_s = relu(cat(hf,ht) @ W1.T + b1)  (s=0) / cat(ht,hf) (s=1)   # [E, 256]
  p_s = z_s @ W2.T + b2                                           # [E, 96]
  p_s -> [E, R=4, 2, A=12];  payoff_s[e,i,j] = sum_r p_s[e,r,0,i]*p_s[e,r,1,j]
  out[b,e] = 0.5*(payoff_0[e] + payoff_1[e].T)                    # [E, 12, 12]

Key factorization: cat(hf,ht) @ W1.T = U[from_e] + V[to_e] with
  U = hidden @ W1[:, :DH].T, V = hidden @ W1[:, DH:].T  (per-agent, N=32 agents)
so layer 1 runs on 32 agents instead of 2*496 edge-dirs (31x less compute),
and the edge gather becomes a tiny one-hot matmul (edges known at trace time).

Sharding: data-parallel over batch; 32 batches per core on 8 cores.
"""

import numpy as np
from contextlib import ExitStack

import concourse.bacc as bacc
import concourse.bass as bass
import concourse.tile as tile
import concourse.mybir as mybir
from concourse.bass_utils import run_bass_kernel_spmd
from concourse.masks import make_identity

F32 = mybir.dt.float32
AF = mybir.ActivationFunctionType
ALU = mybir.AluOpType

B, N, DH = 256, 32, 128
E = 496
DIM_HID = 256
A = 12
R = 4
PO = 2 * R * A  # 96
NCORES = 8
BL = B // NCORES  # 32 batches per core
ET = 124          # edge tile (E = 4*124)
NET = E // ET     # 4


def _build_consts(W1, b1, W2, b2, edges_from, edges_to):
    """Host-side constant tensors derived from weights + edge lists."""
    ef = np.asarray(edges_from).astype(np.int64)
    et = np.asarray(edges_to).astype(np.int64)
    W1 = np.asarray(W1, dtype=np.float32)
    W2 = np.asarray(W2, dtype=np.float32)
    b1 = np.asarray(b1, dtype=np.float32)
    b2 = np.asarray(b2, dtype=np.float32)

    # [128 dh, 512] : cols 0:256 -> W1a^T (U), cols 256:512 -> W1b^T (V)
    w1t_ab = np.concatenate([W1[:, :DH].T, W1[:, DH:].T], axis=1).copy()

    # selection matrix [64, 2*E]: z[:, s*E + e] = U[sel_u(s,e)] + V[sel_v(s,e)]
    sel = np.zeros((64, 2 * E), dtype=np.float32)
    for e in range(E):
        sel[ef[e], e] = 1.0            # s=0: U[from]
        sel[32 + et[e], e] = 1.0       # s=0: V[to]
        sel[et[e], E + e] = 1.0        # s=1: U[to]
        sel[32 + ef[e], E + e] = 1.0   # s=1: V[from]

    # scale the p0 half (half-index 0 of the (r, half, a) output dims) by 0.5
    # so the final combine is just payoff0 + payoff1^T.
    W2s = W2.copy()
    b2s = b2.copy()
    p0_rows = np.concatenate([np.arange(r * 2 * A, r * 2 * A + A) for r in range(R)])
    W2s[p0_rows, :] *= 0.5
    b2s[p0_rows] *= 0.5
    w2t = W2s.T.copy()          # [256, 96]
    b2c = b2s[:, None].copy()   # [96, 1]
    b1c = np.stack([b1[:DH], b1[DH:]], axis=1).copy()  # [128, 2]

    return {
        "w1t_ab": w1t_ab,   # [128, 512]
        "sel": sel,         # [64, 992]
        "w2t": w2t,         # [256, 96]
        "b1c": b1c,         # [128, 2]
        "b2c": b2c,         # [96, 1]
    }


def build_kernel(ctx: ExitStack, tc: tile.TileContext, outs, ins):
    """Tile kernel: ins/outs are dicts of DRAM APs.

    ins: hidden [BL*N, DH], w1t_ab [128,512], sel [64,992], w2t [256,96],
         b1c [128,2], b2c [96,1]
    outs: out [BL*E, A*A]
    """
    nc = tc.nc
    hid_d = ins["hidden"]
    out_d = outs["out"]

    consts = ctx.enter_context(tc.tile_pool(name="consts", bufs=1))
    hidT_pool = ctx.enter_context(tc.tile_pool(name="hidT", bufs=1))
    nat_pool = ctx.enter_context(tc.tile_pool(name="nat", bufs=2))
    uv_pool = ctx.enter_context(tc.tile_pool(name="uv", bufs=3))
    h_pool = ctx.enter_context(tc.tile_pool(name="h", bufs=3))
    p_pool = ctx.enter_context(tc.tile_pool(name="p", bufs=3))
    pT_pool = ctx.enter_context(tc.tile_pool(name="pT", bufs=3))
    pay_pool = ctx.enter_context(tc.tile_pool(name="pay", bufs=3))
    tmp_pool = ctx.enter_context(tc.tile_pool(name="tmp", bufs=3))
    outp_pool = ctx.enter_context(tc.tile_pool(name="outp", bufs=4))

    ps_small = ctx.enter_context(tc.tile_pool(name="ps_small", bufs=2, space="PSUM"))
    ps_uv = ctx.enter_context(tc.tile_pool(name="ps_uv", bufs=2, space="PSUM"))
    ps_z = ctx.enter_context(tc.tile_pool(name="ps_z", bufs=2, space="PSUM"))
    ps_p = ctx.enter_context(tc.tile_pool(name="ps_p", bufs=2, space="PSUM"))

    # ---- load constants ----
    w1t = consts.tile([128, 512], F32)
    nc.sync.dma_start(w1t[:], ins["w1t_ab"][:])
    sel = consts.tile([64, 2 * E], F32)
    nc.sync.dma_start(sel[:], ins["sel"][:])
    w2t = consts.tile([128, 2, PO], F32)  # [128, (k, 96)]
    nc.sync.dma_start(w2t[:, 0, :], ins["w2t"][0:128, :])
    nc.sync.dma_start(w2t[:, 1, :], ins["w2t"][128:256, :])
    b1c = consts.tile([128, 2], F32)
    nc.sync.dma_start(b1c[:], ins["b1c"][:])
    b2c = consts.tile([PO, 1], F32)
    nc.sync.dma_start(b2c[:], ins["b2c"][:])
    ident = consts.tile([128, 128], F32)
    make_identity(nc, ident)

    # ---- stage A: transpose hidden -> hidT [128 dh, (b,n)=1024] ----
    hidT = hidT_pool.tile([128, BL * N], F32)
    for g in range(BL * N // 128):
        nat = nat_pool.tile([128, 128], F32)
        nc.sync.dma_start(nat[:], hid_d[g * 128:(g + 1) * 128, :])
        pst = ps_small.tile([128, 128], F32, tag="ps_small")
        nc.tensor.transpose(pst[:], nat[:], ident[:])
        nc.any.tensor_copy(hidT[:, g * 128:(g + 1) * 128], pst[:])

    for b in range(BL):
        # ---- layer 1: UV [64=(uv,n), 256] ----
        psuv = ps_uv.tile([64, DIM_HID], F32)
        lhs_h = hidT[:, b * N:(b + 1) * N]
        nc.tensor.matmul(psuv[0:32, :], lhs_h, w1t[:, 0:256], start=True, stop=True)
        nc.tensor.matmul(psuv[32:64, :], lhs_h, w1t[:, 256:512], start=True, stop=True)
        uv = uv_pool.tile([64, DIM_HID], F32)
        nc.any.tensor_copy(uv[:], psuv[:])

        # ---- z = UV-gather, relu(z + b1) -> h[half] [128, 992] ----
        h = [h_pool.tile([128, 2 * E], F32, tag=f"h{half}", name=f"h{half}")
             for half in range(2)]
        for half in range(2):
            for sc in range(2):
                psz = ps_z.tile([128, E], F32)
                nc.tensor.matmul(psz[:], uv[:, half * 128:(half + 1) * 128],
                                 sel[:, sc * E:(sc + 1) * E], start=True, stop=True)
                dst = h[half][:, sc * E:(sc + 1) * E]
                if (half + sc) % 2 == 0:
                    nc.vector.tensor_scalar(dst, psz[:], b1c[:, half:half + 1], 0.0,
                                            op0=ALU.add, op1=ALU.max)
                else:
                    nc.scalar.activation(dst, psz[:], AF.Relu,
                                         bias=b1c[:, half:half + 1])

        # ---- layer 2: p [96, 992] (+b2, p0-half prescaled 0.5 via weights) ----
        p_sb = p_pool.tile([PO, 2 * E], F32)
        for sc in range(2):
            psp = ps_p.tile([PO, E], F32)
            nc.tensor.matmul(psp[:], w2t[:, 0, :], h[0][:, sc * E:(sc + 1) * E],
                             start=True, stop=False)
            nc.tensor.matmul(psp[:], w2t[:, 1, :], h[1][:, sc * E:(sc + 1) * E],
                             start=False, stop=True)
            dst = p_sb[:, sc * E:(sc + 1) * E]
            if sc == 0:
                nc.vector.tensor_scalar_add(dst, psp[:], b2c[:])
            else:
                nc.scalar.add(dst, psp[:], b2c[:])

        # ---- transpose p -> pT [124, (s,et), 96] ----
        pT = pT_pool.tile([ET, 2 * NET, PO], F32)
        for t in range(2 * NET):
            pst = ps_small.tile([128, 128], F32, tag="ps_small")
            nc.tensor.transpose(pst[0:ET, 0:PO], p_sb[:, t * ET:(t + 1) * ET],
                                ident[0:PO, 0:PO])
            if t % 2 == 0:
                nc.vector.tensor_copy(pT[:, t, :], pst[0:ET, 0:PO])
            else:
                nc.scalar.copy(pT[:, t, :], pst[0:ET, 0:PO])

        # ---- einsum + combine per edge tile ----
        for et in range(NET):
            pay = pay_pool.tile([ET, 2, A * A], F32)
            for s in range(2):
                src = pT[:, s * NET + et, :]  # [124, 96] cols (r, half, a)
                eng = nc.vector if s == 0 else nc.gpsimd
                tmp = tmp_pool.tile([ET, A * A], F32, tag=f"tmp{s}")
                acc = pay[:, s, :].rearrange("p (i j) -> p i j", i=A)
                tmp3 = tmp.rearrange("p (i j) -> p i j", i=A)
                for r in range(R):
                    p0 = src[:, r * 2 * A: r * 2 * A + A]        # [124, 12] (x0.5)
                    p1 = src[:, r * 2 * A + A: r * 2 * A + 2 * A]
                    p0b = p0.unsqueeze(2).broadcast_to([ET, A, A])
                    p1b = p1.unsqueeze(1).broadcast_to([ET, A, A])
                    dst3 = acc if r == 0 else tmp3
                    eng.tensor_mul(dst3, p0b, p1b)
                    if r > 0:
                        eng.tensor_add(acc, acc, tmp3)
            # out = pay0 + pay1^T  (both already carry the 0.5)
            o = outp_pool.tile([ET, A * A], F32)
            o3 = o.rearrange("p (i j) -> p i j", i=A)
            pay0 = pay[:, 0, :].rearrange("p (i j) -> p i j", i=A)
            pay1T = pay[:, 1, :].rearrange("p (j i) -> p i j", j=A)
            nc.vector.tensor_add(o3, pay0, pay1T)
            r0 = b * E + et * ET
            nc.sync.dma_start(out_d[r0:r0 + ET, :], o[:])


def _shard_inputs(inputs):
    """Build the 8 per-core input maps (host-side)."""
    hidden = np.asarray(inputs["hidden_states_n"], dtype=np.float32)
    consts = _build_consts(inputs["W1"], inputs["b1"], inputs["W2"], inputs["b2"],
                           inputs["edges_from"], inputs["edges_to"])
    in_maps = []
    for c in range(NCORES):
        m = {"hidden": hidden[c * BL:(c + 1) * BL].reshape(BL * N, DH).copy()}
        m.update({k: v for k, v in consts.items()})
        in_maps.append(m)
    return in_maps


def _build_program():
    nc = bacc.Bacc("TRN2", target_bir_lowering=False, debug=False,
                   enable_asserts=False)
    ins = {
        "hidden": nc.dram_tensor("hidden", [BL * N, DH], F32,
                                 kind="ExternalInput").ap(),
        "w1t_ab": nc.dram_tensor("w1t_ab", [128, 512], F32,
                                 kind="ExternalInput").ap(),
        "sel": nc.dram_tensor("sel", [64, 2 * E], F32,
                              kind="ExternalInput").ap(),
        "w2t": nc.dram_tensor("w2t", [256, PO], F32,
                              kind="ExternalInput").ap(),
        "b1c": nc.dram_tensor("b1c", [128, 2], F32, kind="ExternalInput").ap(),
        "b2c": nc.dram_tensor("b2c", [PO, 1], F32, kind="ExternalInput").ap(),
    }
    outs = {
        "out": nc.dram_tensor("out", [BL * E, A * A], F32,
                              kind="ExternalOutput").ap(),
    }
    with tile.TileContext(nc) as tc:
        with ExitStack() as stack:
            build_kernel(stack, tc, outs, ins)
    nc.compile()
    return nc


_PROGRAM_CACHE = {}


def kernel(**inputs) -> np.ndarray:
    if "nc" not in _PROGRAM_CACHE:
        _PROGRAM_CACHE["nc"] = _build_program()
    nc = _PROGRAM_CACHE["nc"]
    in_maps = _shard_inputs(inputs)
    res = run_bass_kernel_spmd(nc, in_maps, core_ids=list(range(NCORES)))
    out = np.concatenate(
        [res.results[c]["out"].reshape(BL, E, A, A) for c in range(NCORES)], axis=0
    )
    return out.astype(np.float32)


if __name__ == "__main__":
    import reference
    inputs = {k: np.asarray(v) for k, v in reference.setup_inputs().items()}
    got = kernel(**inputs)
    exp = np.asarray(reference.reference(**inputs))
    err = np.abs(got - exp).max() / np.abs(exp).max()
    print("Relative error:", err)
